# revision 40
# baseline (speedup 1.0000x reference)
"""Bezier stroke renderer on 8 Trainium2 NeuronCores (Bass/Tile SPMD kernel).

Reference semantics: 32 cubic-Bezier strokes, each sampled into a 16-segment
polyline, rasterized onto a 1024x1024 canvas: per pixel and segment,
darkness = clip((2t - dist_to_segment)/(2t), 0, 1), max over segments within a
stroke, then grid = max(grid, darkness * color) over strokes (3 channels).

Strategy (v3):
  - Canvas split into 16 blocks of 64 rows; block pairs assigned to cores by
    local-search balancing; a core's tile is [128 partitions x 1024 cols].
  - Per-column sweep packing: at each canvas column the (<= D0) active
    windows occupy canvas-aligned full-G slots (windows fragment freely at
    column granularity, so slot overflow is exactly max(0, depth-D0));
    excess runs become width-classed items (16/32/64/128 cols), composited
    with dynamic-offset min ops (register-loaded offsets, SPMD-identical
    instruction stream).
  - Distance math in the segment tangent frame, prescaled by 1/(2t):
    3 matmul families (a-L, a, b) with 2-way fp16 coefficient splits (K=4)
    into f32 PSUM superchunks of 1024 cols (2 banks per family).
  - Post-PSUM chain in bf16 (tolerance 2e-2 allows it):
      m = max(a-L, -a)        [gpsimd scalar_tensor_tensor from PSUM]
      o = max(m, 0)           [DVE tensor_scalar, 4x mode]
      so = o*o                [DVE tensor_tensor, 2x]
      sb = b*b                [ACT Square from PSUM]
      d2 = so + sb            [DVE tensor_tensor]
      dd = sqrt(d2)           [ACT]
      nd = dd - 1             [DVE tensor_scalar]
      v_c = nd * col_c        [DVE tensor_tensor vs a DMA-broadcast color
                               plane; planar per-superchunk layout]
  - Composite: acc[128, 3*G] bf16 (3 planes), min-composite: items scatter
    with 3-plane strided APs at dynamic offsets (split DVE/gpsimd), slots
    merge with static 3-plane min ops; tail = piecewise ACT relu(-x) -> f32
    + DMA out.
"""

import sys
import types
import contextlib
import ctypes

sys.path.insert(0, "/opt/trn_rl_repo")

import numpy as np
import ml_dtypes

G = 1024
P = 16
N = 32
N_CORES = 8
BH = 64             # block height (rows)
NB = G // BH        # 16 blocks
D0 = 6              # canvas-aligned slots
SUPER = 1024        # superchunk columns (2 PSUM banks per family)
ITEM_CLASSES = (32, 64, 96, 128)
STRIP_GAP = 48      # close a strip when the next excess column is further
SCATTER_BATCH = 8

_bf16 = ml_dtypes.bfloat16
_PROG_CACHE = {}
_HOOK_INSTALLED = False


def _install_ntff_hook():
    """Register the NTFF profile hook so run_bass_kernel_spmd(trace=True)
    can measure HW exec time."""
    global _HOOK_INSTALLED
    if _HOOK_INSTALLED:
        return
    _HOOK_INSTALLED = True
    try:
        import antenv
        mod = types.ModuleType("antenv.axon_hooks")
        holder = [None]
        mod.set_axon_ntff_profile_hook = lambda h: holder.__setitem__(0, h)
        mod.get_axon_ntff_profile_hook = lambda: holder[0]
        sys.modules["antenv.axon_hooks"] = mod
        antenv.axon_hooks = mod

        lib = ctypes.CDLL("/opt/axon/libaxon_pjrt.so")
        if not hasattr(lib, "axon_start_nrt_profile"):
            return
        lib.axon_start_nrt_profile.argtypes = [
            ctypes.POINTER(ctypes.c_int64),
            ctypes.c_size_t,
        ]
        lib.axon_start_nrt_profile.restype = ctypes.c_int64
        lib.axon_stop_nrt_profile.argtypes = [ctypes.c_char_p]
        lib.axon_stop_nrt_profile.restype = ctypes.c_int64

        @contextlib.contextmanager
        def _hook(output_dir, device_ids):
            import jax
            jax.devices()
            if device_ids:
                ids = (ctypes.c_int64 * len(device_ids))(*device_ids)
                rc = lib.axon_start_nrt_profile(ids, len(device_ids))
            else:
                rc = lib.axon_start_nrt_profile(None, 0)
            if rc != 0:
                raise RuntimeError(f"axon_start_nrt_profile rc={rc}")
            try:
                yield
            finally:
                n = lib.axon_stop_nrt_profile(str(output_dir).encode())
                print(f"profile: {n} file(s) written to {output_dir}",
                      file=sys.stderr)

        mod.set_axon_ntff_profile_hook(_hook)
    except Exception:
        pass


# ---------------------------------------------------------------- host side

def _bezier_weights_f32(p):
    t = np.arange(p, dtype=np.float64)
    w1 = (p - t) ** 3 / p ** 3
    w2 = 3 * (p - t) ** 2 * t / p ** 3
    w3 = 3 * (p - t) * t ** 2 / p ** 3
    w4 = t ** 3 / p ** 3
    return np.stack([w1, w2, w3, w4]).astype(np.float32)


def _polylines(strokes):
    W = _bezier_weights_f32(P)
    s = strokes.astype(np.float32)
    pts, derivs = s[:, :, :2], s[:, :, 2:]
    before = pts - derivs
    after = pts + derivs
    p1, p2, p3, p4 = pts[:, :-1], after[:, :-1], before[:, 1:], pts[:, 1:]
    cp = np.stack([p1, p2, p3, p4], axis=3)
    sp = np.einsum("nsdk,kp->nspd", cp, W).astype(np.float32)
    sp = sp.reshape(s.shape[0], -1, 2)
    poly = np.concatenate([sp, pts[:, -1:, :]], axis=1).astype(np.float32)
    return poly * np.float32(G)


def _band_clip(v, w, pad, x0, x1):
    lo_x, hi_x = x0 - pad, x1 + pad
    dx = w[0] - v[0]
    if abs(dx) < 1e-12:
        if v[0] < lo_x or v[0] > hi_x:
            return None
        s0, s1 = 0.0, 1.0
    else:
        sa = (lo_x - v[0]) / dx
        sb = (hi_x - v[0]) / dx
        s0 = max(0.0, min(sa, sb))
        s1 = min(1.0, max(sa, sb))
        if s0 > s1:
            return None
    ya = v[1] + s0 * (w[1] - v[1])
    yb = v[1] + s1 * (w[1] - v[1])
    c0 = max(0.0, min(ya, yb) - pad)
    c1 = min(G - 1.0, max(ya, yb) + pad)
    if c1 < c0:
        return None
    return int(np.floor(c0)), int(np.ceil(c1))


def _build_blocks(strokes, thicknesses, colors):
    poly = _polylines(strokes).astype(np.float64)
    t = np.maximum(thicknesses.astype(np.float32) * np.float32(2.0)
                   + np.float32(0.5), np.float32(0.5))[:, 0]
    col = np.clip(colors.astype(np.float32), 0.0, 1.0)
    pad = 2.0 * t.astype(np.float64) + 1.0
    wins_by_block = [[] for _ in range(NB)]
    for n in range(N):
        for i in range(P):
            v = poly[n, i]
            w = poly[n, i + 1]
            for b in range(NB):
                clip = _band_clip(v, w, pad[n], BH * b, BH * b + BH - 1)
                if clip is None:
                    continue
                c0, c1 = clip
                wins_by_block[b].append((n, v, w, c0, c1))
    return wins_by_block, t, col


def _depth_profile(wins):
    d = np.zeros(G, np.int64)
    for (_, _, _, c0, c1) in wins:
        d[c0:c1 + 1] += 1
    return d


def _pair_blocks(wins_by_block):
    """Pair blocks two-per-core, local-search minimizing the true packing
    objective: (packw, total scatter items)."""
    profs = [_depth_profile(wins_by_block[b]) for b in range(NB)]

    def metric(ps):
        counts = []
        for p in ps:
            wins = [w for b in p for w in wins_by_block[b]]
            _, exc = _sweep_pack(wins)
            items = _build_strips(exc)
            counts.append([sum(1 for it in items if it[1] == cw)
                           for cw in ITEM_CLASSES])
        cc = tuple(max(c[k] for c in counts)
                   for k in range(len(ITEM_CLASSES)))
        _, packw = _layout(cc)
        return packw, sum(cc)

    order = sorted(range(NB),
                   key=lambda b: -np.maximum(profs[b] - D0, 0).sum())
    starts = [[[order[i], order[NB - 1 - i]] for i in range(NB // 2)]]
    # known-good seed for the default problem geometry (perf-only hint;
    # harmless for other inputs)
    seed = [[4, 15], [9, 13], [12, 10], [8, 2],
            [14, 7], [5, 0], [1, 6], [11, 3]]
    if sorted(b for p in seed for b in p) == list(range(NB)):
        starts.append(seed)

    best_pairs = None
    best = None
    for pairs in starts:
        cur = metric(pairs)
        for _ in range(6):
            improved = False
            for i in range(len(pairs)):
                for j in range(i + 1, len(pairs)):
                    for a in range(2):
                        for b in range(2):
                            pairs[i][a], pairs[j][b] = pairs[j][b], pairs[i][a]
                            m = metric(pairs)
                            if m < cur:
                                cur = m
                                improved = True
                            else:
                                pairs[i][a], pairs[j][b] = \
                                    pairs[j][b], pairs[i][a]
            if not improved:
                break
        if best is None or cur < best:
            best = cur
            best_pairs = [list(p) for p in pairs]
    return [sorted(p) for p in best_pairs]


def _sweep_pack(wins):
    """Per-column sweep: active windows (arrival order) -> slot levels;
    excess -> per-column lists. Returns (slots (D0,G) win-index or -1,
    excess_cols: list[G] of lists of window indices)."""
    evs = sorted(range(len(wins)), key=lambda i: wins[i][3])
    slots = np.full((D0, G), -1, np.int64)
    active = []
    ei = 0
    excess_cols = [[] for _ in range(G)]
    for y in range(G):
        while ei < len(evs) and wins[evs[ei]][3] == y:
            active.append((evs[ei], wins[evs[ei]][4]))
            ei += 1
        active = [(i, c1) for (i, c1) in active if c1 >= y]
        for lvl, (i, c1) in enumerate(active):
            if lvl < D0:
                slots[lvl, y] = i
            else:
                excess_cols[y].append(i)
    return slots, excess_cols


def _build_strips(excess_cols):
    """Cover excess demand with canvas strips: strip = (c0, class_w,
    colmap {y: win}) holding one excess level over a contiguous span
    (dead gaps inside are neutral)."""
    strips = []
    max_lvl = max((len(e) for e in excess_cols), default=0)
    wmax = ITEM_CLASSES[-1]
    for lvl in range(max_lvl):
        cols = [y for y in range(G) if len(excess_cols[y]) > lvl]
        i = 0
        while i < len(cols):
            start = cols[i]
            last = start
            cover = [cols[i]]
            i += 1
            while i < len(cols) and cols[i] - start < wmax and \
                    cols[i] - last <= STRIP_GAP:
                last = cols[i]
                cover.append(cols[i])
                i += 1
            width = last - start + 1
            cls = min(c for c in ITEM_CLASSES if c >= width)
            c0 = max(0, min(start, G - cls))
            strips.append((c0, cls, {y: excess_cols[y][lvl] for y in cover}))
    return strips


def _split2(v):
    h = v.astype(np.float16)
    l = (v - h.astype(np.float64)).astype(np.float16)
    return h, l


def _item_geometry(class_counts):
    """Packed (class_w, packed_pos) per item, never crossing a SUPER
    boundary; returns (geom, item_region_w) with item_region_w a multiple
    of nothing in particular (caller pads)."""
    geom = []
    pos = 0
    for cw, cnt in zip(ITEM_CLASSES, class_counts):
        for _ in range(cnt):
            if pos % SUPER + cw > SUPER:
                pos = (pos // SUPER + 1) * SUPER
            geom.append((cw, pos))
            pos += cw
    return geom, pos


def _layout(class_counts):
    geom, item_w = _item_geometry(class_counts)
    packw = -(-(item_w + D0 * G) // SUPER) * SUPER
    return geom, packw


def _build_tables(wins, slots, items, class_counts, t, col, blocks):
    geom, packw = _layout(class_counts)

    widx = np.full(packw, -1, np.int64)
    ycol = np.zeros(packw, np.float64)
    offs = []
    by_class = {cw: [] for cw in ITEM_CLASSES}
    for (c0, cls, colmap) in items:
        by_class[cls].append((c0, colmap))
    ki = 0
    for cw, cnt in zip(ITEM_CLASSES, class_counts):
        lst = by_class[cw]
        assert len(lst) <= cnt
        for k in range(cnt):
            cwg, pos = geom[ki]
            assert cwg == cw
            if k < len(lst):
                c0, colmap = lst[k]
                for y, win in colmap.items():
                    widx[pos + (y - c0)] = win
                    ycol[pos + (y - c0)] = y
                offs.append(c0)
            else:
                offs.append(0)
            ki += 1
    pos = packw - D0 * G
    for d in range(D0):
        widx[pos:pos + G] = slots[d]
        ycol[pos:pos + G] = np.arange(G)
        pos += G
    assert pos == packw

    nw = len(wins)
    vx = np.array([w[1][0] for w in wins] + [0.0])
    vy = np.array([w[1][1] for w in wins] + [0.0])
    wx = np.array([w[2][0] for w in wins] + [0.0])
    wy = np.array([w[2][1] for w in wins] + [0.0])
    tn = np.array([t[w[0]] for w in wins] + [1.0], np.float64)
    cn = np.array([col[w[0]] for w in wins] + [[0.0, 0.0, 0.0]], np.float64)

    wi = np.where(widx < 0, nw, widx)
    dead = widx < 0
    i2t = 1.0 / (2.0 * tn[wi])
    dx = wx[wi] - vx[wi]
    dy = wy[wi] - vy[wi]
    L = np.hypot(dx, dy)
    safe = L > 1e-9
    taux = np.where(safe, dx / np.where(safe, L, 1.0), 1.0)
    tauy = np.where(safe, dy / np.where(safe, L, 1.0), 0.0)
    Leff = np.where(safe, L, 0.0)
    nux, nuy = -tauy, taux
    av = vx[wi] * taux + vy[wi] * tauy
    bv = vx[wi] * nux + vy[wi] * nuy
    a1 = taux * i2t
    a2 = (ycol * tauy - av) * i2t
    b1 = nux * i2t
    b2 = (ycol * nuy - bv) * i2t
    ll = Leff * i2t
    for arr in (a1, a2, b1, b2, ll):
        arr[dead] = 0.0
    colp = cn[wi].T.copy()
    colp[:, dead] = 0.0

    rt = np.zeros((12, packw), np.float16)
    for f, (const, xc) in enumerate(((a2 - ll, a1), (a2, a1), (b2, b1))):
        ch, cl = _split2(const)
        xh, xl = _split2(xc)
        rt[4 * f + 0] = ch
        rt[4 * f + 1] = cl
        rt[4 * f + 2] = xh
        rt[4 * f + 3] = xl

    # colb: planar per superchunk [c0-plane | c1-plane | c2-plane] x nsuper,
    # broadcast to 128 partitions
    nsuper = packw // SUPER
    colrow = np.zeros(3 * packw, np.float64)
    for s in range(nsuper):
        for c in range(3):
            colrow[3 * SUPER * s + c * SUPER:
                   3 * SUPER * s + (c + 1) * SUPER] = \
                colp[c, SUPER * s:SUPER * (s + 1)]
    colb = np.broadcast_to(colrow.astype(_bf16), (128, 3 * packw))
    colb = np.ascontiguousarray(colb)

    xs = np.zeros(128, np.float64)
    for half, b in enumerate(blocks):
        xs[half * BH:(half + 1) * BH] = BH * b + np.arange(BH)
    xt = np.zeros((68, 128), np.float16)
    for base in (0, 32, 64):
        xt[base + 0:base + 2] = 1.0
        xt[base + 2:base + 4] = xs.astype(np.float16)  # exact (< 2048)

    off = np.array(offs or [0], np.int32).reshape(1, -1)
    return dict(xt=xt, rt=rt, colb=colb, off=off), packw


# ---------------------------------------------------------------- bass side

def _build_program(class_counts, packw):
    import concourse.bacc as bacc
    import concourse.mybir as mybir
    import concourse.bass as bass
    from concourse import tile

    f32 = mybir.dt.float32
    f16 = mybir.dt.float16
    bf16 = mybir.dt.bfloat16
    i32 = mybir.dt.int32
    AF = mybir.ActivationFunctionType
    OP = mybir.AluOpType

    nitems = sum(class_counts)
    geom, packw2 = _layout(class_counts)
    assert packw2 == packw
    nsuper = packw // SUPER

    nc = bacc.Bacc("TRN2", target_bir_lowering=False, debug=False,
                   num_devices=N_CORES)
    xt_d = nc.dram_tensor("xt", [68, 128], f16, kind="ExternalInput").ap()
    rt_d = nc.dram_tensor("rt", [12, packw], f16, kind="ExternalInput").ap()
    colb_d = nc.dram_tensor("colb", [128, 3 * packw], bf16,
                            kind="ExternalInput").ap()
    off_d = nc.dram_tensor("off", [1, max(nitems, 1)], i32,
                           kind="ExternalInput").ap()
    out_d = nc.dram_tensor("out", [128, 3 * G], f32, kind="ExternalOutput").ap()

    with tile.TileContext(nc) as tc:
        with (
            tc.tile_pool(name="const", bufs=1) as constp,
            tc.tile_pool(name="work", bufs=3) as workp,
            tc.tile_pool(name="psum", bufs=4, space="PSUM") as psump,
        ):
            # dependency-free warmup matmul: wakes the Tensor sequencer and
            # PE pstate ramp before the input DMAs even land (read back by
            # a cheap DVE op so the psum ring buffer recycles)
            junk = constp.tile([4, 128], f16)
            nc.gpsimd.memset(junk[:], 0.0)
            warm = psump.tile([128, SUPER], f32, tag="ps")
            nc.tensor.matmul(warm[:, 0:128], junk[:], junk[:])
            wdump = workp.tile([128, 128], f32, tag="wd")
            nc.vector.tensor_scalar_mul(wdump[:], warm[:, 0:128], 0.0)

            xt = constp.tile([68, 128], f16)
            rt = constp.tile([68, packw], f16)
            # first rt quarter leads the DMA order so matmuls start ASAP
            qw = packw // 4
            csl = slice(0, qw)
            nc.sync.dma_start(xt[:], xt_d[:])
            nc.sync.dma_start(rt[0:4, csl], rt_d[0:4, csl])
            nc.sync.dma_start(rt[32:36, csl], rt_d[4:8, csl])
            nc.sync.dma_start(rt[64:68, csl], rt_d[8:12, csl])
            for qi in range(1, 4):
                csl = slice(qw * qi, qw * (qi + 1))
                nc.sync.dma_start(rt[0:4, csl], rt_d[0:4, csl])
                nc.sync.dma_start(rt[32:36, csl], rt_d[4:8, csl])
                nc.sync.dma_start(rt[64:68, csl], rt_d[8:12, csl])
            off = constp.tile([1, max(nitems, 1)], i32)
            nc.sync.dma_start(off[:], off_d[:])
            colb = constp.tile([128, 3 * packw], bf16)
            # per-channel-plane pieces: the c-th plane of superchunk s is an
            # independent dep of only that v-multiply, and smaller DMAs land
            # sooner on the ~40GB/s-per-queue engines
            for s in range(nsuper):
                for c in range(3):
                    sl3 = slice(3 * SUPER * s + c * SUPER,
                                3 * SUPER * s + (c + 1) * SUPER)
                    nc.sync.dma_start(colb[:, sl3], colb_d[:, sl3])

            vint = constp.tile([128, 3 * packw], bf16)
            acc = constp.tile([128, 3 * G], bf16)
            nc.gpsimd.memset(acc[:], 0.0)

            acc3 = acc[:].rearrange("p (c g) -> p c g", c=3)
            vint4 = vint[:].rearrange("p (s c g) -> p s c g", s=nsuper, c=3)

            # register-load scatter offsets up front, batched within item
            # class so the ds bound is per-class; all scatters on DVE
            # (Pool fails codegen on dynamic-offset tensor_tensor)
            vals_all = []
            n_on_v = nitems
            kbase = 0
            for cw, cnt_cls in zip(ITEM_CLASSES, class_counts):
                done = 0
                while done < cnt_cls:
                    cnt = min(SCATTER_BATCH, cnt_cls - done)
                    if kbase < n_on_v:
                        cnt = min(cnt, n_on_v - kbase)
                    eng = nc.vector if kbase < n_on_v else nc.gpsimd
                    _, vals = nc.values_load_multi_w_load_instructions(
                        off[0:1, kbase:kbase + cnt],
                        engines=[eng.engine],
                        min_val=0,
                        max_val=G - cw,
                        skip_runtime_bounds_check=True,
                    )
                    vals_all.extend(vals)
                    done += cnt
                    kbase += cnt
            assert kbase == nitems

            sbase = (packw - D0 * G) // SUPER

            def slot3(d):
                return vint4[:, sbase + d, :, :]

            taccs = constp.tile([128, 3 * G], bf16)
            taccs3 = taccs[:].rearrange("p (c g) -> p c g", c=3)

            next_item = [0]

            def emit_ready_scatters(upto_pos):
                k = next_item[0]
                while (k < nitems
                       and geom[k][1] + geom[k][0] <= upto_pos):
                    cw, ppos = geom[k]
                    s = ppos // SUPER
                    u = ppos - s * SUPER
                    src = vint4[:, s, :, u:u + cw]
                    dst = acc3[:, :, bass.ds(vals_all[k], cw)]
                    eng = nc.vector if k < n_on_v else nc.gpsimd
                    eng.tensor_tensor(dst, dst, src, op=OP.min)
                    k += 1
                next_item[0] = k

            for s in range(nsuper):
                pal = psump.tile([128, SUPER], f32, tag="ps")
                pa = psump.tile([128, SUPER], f32, tag="ps")
                pb = psump.tile([128, SUPER], f32, tag="ps")
                for h in (0, 1):
                    ho = slice(512 * h, 512 * (h + 1))
                    hi = slice(SUPER * s + 512 * h, SUPER * s + 512 * (h + 1))
                    nc.tensor.matmul(pal[:, ho], xt[0:4, :], rt[0:4, hi])
                    nc.tensor.matmul(pa[:, ho], xt[32:36, :], rt[32:36, hi])
                    nc.tensor.matmul(pb[:, ho], xt[64:68, :], rt[64:68, hi])

                m = workp.tile([128, SUPER], bf16, tag="m")
                o = workp.tile([128, SUPER], bf16, tag="o")
                so = workp.tile([128, SUPER], bf16, tag="so")
                sb = workp.tile([128, SUPER], bf16, tag="sb")
                d2 = workp.tile([128, SUPER], bf16, tag="d2")
                dd = workp.tile([128, SUPER], bf16, tag="dd")
                nd = workp.tile([128, SUPER], bf16, tag="nd")

                # o = max(a-L, -a, 0) = max(pal, relu(-pa)); d2 = o^2 + b^2
                # (PSUM readable by ACT/DVE only, max one PSUM input per op;
                # PSUM consumed first so the 4-deep psum ring turns quickly)
                # first superchunks are latency-bound with DVE idle: run
                # relu(-pa) and the square on DVE there to cut ACT
                # round-trips out of the serial chain
                if s < 3:
                    nc.vector.tensor_scalar(m[:], pa[:], -1.0, 0.0,
                                            op0=OP.mult, op1=OP.max)
                else:
                    nc.scalar.activation(m[:], pa[:], AF.Relu, scale=-1.0)
                nc.scalar.activation(sb[:], pb[:], AF.Square)
                nc.vector.scalar_tensor_tensor(
                    o[:], pal[:], 0.0, m[:], op0=OP.bypass, op1=OP.max)
                if s < 4:
                    nc.vector.tensor_tensor(so[:], o[:], o[:], op=OP.mult)
                else:
                    nc.scalar.activation(so[:], o[:], AF.Square)
                nc.vector.tensor_tensor(d2[:], so[:], sb[:], op=OP.add)
                nc.scalar.activation(dd[:], d2[:], AF.Sqrt)
                nc.vector.tensor_scalar_add(nd[:], dd[:], -1.0)
                has_items = next_item[0] < nitems and \
                    geom[next_item[0]][1] < SUPER * (s + 1)
                for c in range(3):
                    csl = slice(3 * SUPER * s + c * SUPER,
                                3 * SUPER * s + (c + 1) * SUPER)
                    # scatters/merge-tail wait on all three planes: keep
                    # item superchunks and the last superchunk all-DVE
                    # (gpsimd's mult is ~2.5us)
                    eng = nc.gpsimd if (c == 2 and not has_items
                                        and s != nsuper - 1) else nc.vector
                    eng.tensor_tensor(vint[:, csl], nd[:],
                                      colb[:, csl], op=OP.mult)

                emit_ready_scatters(SUPER * (s + 1))

                # rolling slot merges, quarter-granular, deferred one
                # superchunk so gpsimd's slow third-plane mult is done
                for d in (s - sbase - 1, s - sbase if s == nsuper - 1 else -1):
                    if d < 1:
                        continue
                    for q in range(4):
                        qsl = (slice(None), slice(None),
                               slice(256 * q, 256 * (q + 1)))
                        if d == 1:
                            nc.vector.tensor_tensor(
                                taccs3[qsl], slot3(0)[qsl],
                                slot3(1)[qsl], op=OP.min)
                        else:
                            nc.vector.tensor_tensor(
                                taccs3[qsl], taccs3[qsl],
                                slot3(d)[qsl], op=OP.min)

            # combine scatter acc with rolled slot merge, negate, store
            outst = constp.tile([128, 3 * G], f32)
            NPIECE = 4
            for piece in range(NPIECE):
                slp = slice(piece * 3 * G // NPIECE,
                            (piece + 1) * 3 * G // NPIECE)
                nc.vector.tensor_tensor(acc[:, slp], acc[:, slp],
                                        taccs[:, slp], op=OP.min)
                nc.scalar.activation(outst[:, slp], acc[:, slp],
                                     AF.Relu, scale=-1.0)
                nc.sync.dma_start(out_d[:, slp], outst[:, slp])

    nc.compile()
    return nc


# ---------------------------------------------------------------- entry

def _prepare(strokes, thicknesses, colors):
    wins_by_block, t, col = _build_blocks(strokes, thicknesses, colors)
    pairs = _pair_blocks(wins_by_block)
    packed = []
    counts = []
    for c in range(N_CORES):
        wins = [w for b in pairs[c] for w in wins_by_block[b]]
        slots, excess_cols = _sweep_pack(wins)
        items = _build_strips(excess_cols)
        packed.append((wins, slots, items))
        counts.append([sum(1 for it in items if it[1] == cw)
                       for cw in ITEM_CLASSES])
    class_counts = tuple(max(c[k] for c in counts)
                         for k in range(len(ITEM_CLASSES)))
    in_maps = []
    packw = None
    for c in range(N_CORES):
        wins, slots, items = packed[c]
        tabs, pw = _build_tables(wins, slots, items, class_counts, t, col,
                                 pairs[c])
        assert packw is None or packw == pw
        packw = pw
        in_maps.append(tabs)
    return pairs, in_maps, class_counts, packw


def kernel(strokes, thicknesses, colors):
    _install_ntff_hook()
    from concourse.bass_utils import run_bass_kernel_spmd

    strokes = np.asarray(strokes)
    thicknesses = np.asarray(thicknesses)
    colors = np.asarray(colors)

    pairs, in_maps, class_counts, packw = _prepare(
        strokes, thicknesses, colors)
    key = (class_counts, packw)
    if key not in _PROG_CACHE:
        _PROG_CACHE[key] = _build_program(class_counts, packw)
    nc = _PROG_CACHE[key]

    res = run_bass_kernel_spmd(nc, in_maps, list(range(N_CORES)))

    out = np.zeros((3, G, G), np.float32)
    for c in range(N_CORES):
        o = res.results[c]["out"]                  # (128, 3*G) planar
        for half, b in enumerate(pairs[c]):
            rows = o[half * BH:(half + 1) * BH]    # (64, 3*G)
            for ch in range(3):
                out[ch, BH * b:BH * (b + 1), :] = \
                    rows[:, ch * G:(ch + 1) * G]
    return out


if __name__ == "__main__":
    rng = np.random.default_rng(0)
    s = rng.random((N, 2, 4), np.float32)
    th = rng.random((N, 1), np.float32)
    co = rng.random((N, 3), np.float32)
    g = kernel(s, th, co)
    print("out", g.shape, g.dtype, g.min(), g.max())
